# revision 1
# baseline (speedup 1.0000x reference)
"""Causal single-head attention (B=4, T=4096, C=1024, H=128) on 8 Trainium2
NeuronCores.

Sharding: 8 cores = 4 batches x 2 key-parity shards. Each core handles one
batch and the keys in every other 128-block (parity h = core % 2), computing
UN-normalized partial attention (numerator O^T and denominator) for ALL 4096
queries of its batch via unsafe softmax (scores are N(0,1)-bounded, |s| < 8,
so exp never overflows and max-subtraction is unnecessary; partial results
combine exactly by summation across the two cores of a batch).

SPMD uniformity trick: the program is identical on all cores; per-core
differences live entirely in the DATA. The host passes x[b].T with its
columns block-permuted so that this core's keys always sit at the EVEN
128-block positions, plus two per-core causal mask tiles for the two
diagonal blocks of each 512-query group. The host un-permutes the returned
query axis and combines: out = (O0 + O1) / (d0 + d1), transposed.

On-chip layout (everything f32r so matmuls run at full PE rate, N=512):
  S^T blocks [128 keys, 512 queries] = matmul(lhsT=K^T block, rhs=Q^T group)
  E = exp(S^T) on ACT (PSUM->SBUF), masked on DVE for the 2 diagonal blocks
  O^T += matmul(lhsT=V block [k,H], rhs=E)   accumulated in PSUM
  denom = matmul(lhsT=ones[128,1], rhs=sum_i E_i)  (E accumulated on DVE)
"""

import sys
import numpy as np

sys.path.insert(0, "/opt/trn_rl_repo")

B, T, C, H = 4, 4096, 1024, 128
KB = 128            # key block
QG = 512            # query group
NKB = T // KB       # 32 global key blocks
NQG = T // QG       # 8 query groups
NCH = C // 128      # 8 contraction chunks
NST = 4             # supertiles of 1024 positions
SCALE = float(H) ** -0.5

_prog_cache = {}


def _build_program():
    import concourse.mybir as mybir
    import concourse.tile as tile
    from concourse import bacc
    from concourse.masks import make_identity

    F32 = mybir.dt.float32
    F32R = mybir.dt.float32r
    BF16 = mybir.dt.bfloat16
    AF = mybir.ActivationFunctionType

    nc = bacc.Bacc()
    xt = nc.dram_tensor("xt", [C, T], F32R, kind="ExternalInput")
    wq = nc.dram_tensor("wq", [C, H], F32R, kind="ExternalInput")
    wk = nc.dram_tensor("wk", [C, H], F32R, kind="ExternalInput")
    wv = nc.dram_tensor("wv", [C, H], F32R, kind="ExternalInput")
    mp = nc.dram_tensor("mp", [KB, 2 * QG], F32R, kind="ExternalInput")
    ot = nc.dram_tensor("ot", [H, T], F32, kind="ExternalOutput")
    dn = nc.dram_tensor("dn", [1, T], F32, kind="ExternalOutput")

    with tile.TileContext(nc) as tc:
        with (
            tc.tile_pool(name="singles", bufs=1) as singles,
            tc.tile_pool(name="persist", bufs=1) as persist,
            tc.tile_pool(name="xs", bufs=16) as xsp,
            tc.tile_pool(name="epool", bufs=6) as epool,
            tc.tile_pool(name="eacc", bufs=2) as eaccp,
            tc.tile_pool(name="vstage", bufs=2) as vstagep,
            tc.tile_pool(name="outs", bufs=3) as outsp,
            tc.tile_pool(name="pproj", bufs=2, space="PSUM") as pproj,
            tc.tile_pool(name="ps", bufs=4, space="PSUM") as psp,
            tc.tile_pool(name="po", bufs=2, space="PSUM") as pop,
        ):
            # ---- constants ----
            w_sb = {}
            for name, w in (("wq", wq), ("wk", wk), ("wv", wv)):
                t_ = singles.tile([128, NCH, 128], F32R, tag=f"w_{name}")
                nc.sync.dma_start(out=t_, in_=w.rearrange("(c p) h -> p c h", p=128))
                w_sb[name] = t_
            mp_sb = singles.tile([KB, 2 * QG], F32R, tag="mp")
            ones_f = singles.tile([128, 1], F32, tag="ones_f")
            nc.vector.memset(ones_f, 1.0)
            ones_sb = singles.tile([128, 1], F32R, tag="ones")
            nc.scalar.activation(ones_sb, ones_f, AF.Copy)
            ident = singles.tile([128, 128], F32, tag="ident")
            make_identity(nc, ident)

            qT = persist.tile([128, T], F32R, tag="qT")
            kT = persist.tile([128, T // 2], F32R, tag="kT")
            v_sb = persist.tile([128, T // 2], F32R, tag="v")

            def attention_group(j):
                nk = 2 * (j + 1)
                opsum = pop.tile([128, QG], F32, tag="opsum")
                eacc = eaccp.tile([KB, QG], F32R, tag="eacc")
                eacc2 = eaccp.tile([KB, QG], F32R, tag="eacc2")
                qrhs = qT[:, j * QG:(j + 1) * QG]
                es = []

                def emit_pv(i):
                    nc.tensor.matmul(
                        opsum,
                        lhsT=v_sb[:, i * KB:(i + 1) * KB],
                        rhs=es[i],
                        start=(i == 0), stop=(i == nk - 1),
                    )

                for i in range(nk):
                    spsum = psp.tile([KB, QG], F32, tag="spsum")
                    nc.tensor.matmul(
                        spsum,
                        lhsT=kT[:, i * KB:(i + 1) * KB],
                        rhs=qrhs, start=True, stop=True,
                    )
                    # first block's exp lands directly in the accumulator
                    e = eacc if i == 0 else epool.tile([KB, QG], F32R, tag="e")
                    nc.scalar.activation(e, spsum, AF.Exp)
                    if i == nk - 2:
                        nc.vector.tensor_mul(e, e, mp_sb[:, :QG])
                    elif i == nk - 1:
                        nc.vector.tensor_mul(e, e, mp_sb[:, QG:])
                    es.append(e)
                    if i >= 2:
                        emit_pv(i - 2)       # PV lags two stages behind S/exp
                    if i == 1:
                        nc.vector.tensor_copy(eacc2, e)
                    elif i > 1:
                        acc = eacc if i % 2 == 0 else eacc2
                        nc.vector.tensor_add(acc, acc, e)
                emit_pv(nk - 2)
                emit_pv(nk - 1)
                dpsum = pproj.tile([1, QG], F32, tag="proj")
                nc.tensor.matmul(dpsum, lhsT=ones_sb, rhs=eacc,
                                 start=True, stop=False)
                nc.tensor.matmul(dpsum, lhsT=ones_sb, rhs=eacc2,
                                 start=False, stop=True)
                osb = outsp.tile([128, QG], F32, tag="osb")
                nc.vector.tensor_copy(osb, opsum)
                nc.sync.dma_start(out=ot[:, j * QG:(j + 1) * QG], in_=osb)
                dsb = outsp.tile([1, QG], F32, tag="dsb")
                nc.vector.tensor_copy(dsb, dpsum)
                nc.sync.dma_start(out=dn[:, j * QG:(j + 1) * QG], in_=dsb)

            mask_dma_done = []

            for s in range(NST):
                # stream this supertile of x^T: 8 chunk tiles [128, 1024]
                xs = []
                for c in range(NCH):
                    t_ = xsp.tile([128, 1024], F32R, tag="xs")
                    nc.sync.dma_start(
                        out=t_,
                        in_=xt[c * 128:(c + 1) * 128, s * 1024:(s + 1) * 1024],
                    )
                    xs.append(t_)
                if not mask_dma_done:
                    nc.sync.dma_start(out=mp_sb, in_=mp[:])
                    mask_dma_done.append(1)

                def keys_rhs(c):
                    # even 128-blocks of the supertile: cols 0-127, 256-383, ...
                    return xs[c].rearrange(
                        "p (u two b) -> p two u b", two=2, b=128)[:, 0]

                # K^T tile s  [128, 512]
                kpsum = pproj.tile([128, QG], F32, tag="proj")
                for c in range(NCH):
                    nc.tensor.matmul(kpsum, lhsT=w_sb["wk"][:, c], rhs=keys_rhs(c),
                                     start=(c == 0), stop=(c == NCH - 1))
                nc.vector.tensor_copy(kT[:, s * QG:(s + 1) * QG], kpsum)

                # V^T tile s -> transpose into V rows
                vpsum = pproj.tile([128, QG], F32, tag="proj")
                for c in range(NCH):
                    nc.tensor.matmul(vpsum, lhsT=w_sb["wv"][:, c], rhs=keys_rhs(c),
                                     start=(c == 0), stop=(c == NCH - 1))
                vstage = vstagep.tile([128, QG], F32, tag="vstage")
                nc.vector.tensor_copy(vstage, vpsum)
                for u in range(4):
                    tpsum = pproj.tile([128, 128], F32, tag="proj")
                    nc.tensor.transpose(tpsum, vstage[:, u * 128:(u + 1) * 128],
                                        ident)
                    nc.vector.tensor_copy(
                        v_sb[:, (4 * s + u) * 128:(4 * s + u + 1) * 128], tpsum)

                # Q^T tiles 2s, 2s+1 (scale folded in)
                for half in range(2):
                    tq = 2 * s + half
                    qpsum = pproj.tile([128, QG], F32, tag="proj")
                    for c in range(NCH):
                        nc.tensor.matmul(
                            qpsum, lhsT=w_sb["wq"][:, c],
                            rhs=xs[c][:, half * QG:(half + 1) * QG],
                            start=(c == 0), stop=(c == NCH - 1))
                    nc.vector.tensor_copy(qT[:, tq * QG:(tq + 1) * QG], qpsum)

                # attention for the two groups unlocked by this supertile
                attention_group(2 * s)
                attention_group(2 * s + 1)

    nc.finalize()
    return nc


def _get_program():
    if "nc" not in _prog_cache:
        _prog_cache["nc"] = _build_program()
    return _prog_cache["nc"]


def _host_prepare(x, Wq, Wk, Wv):
    """Per-core inputs. Core c: batch b=c//2, parity h=c%2."""
    per_core = []
    for c in range(8):
        b, h = c // 2, c % 2
        pos2glob = np.arange(NKB)
        if h == 1:
            pos2glob = pos2glob.reshape(-1, 2)[:, ::-1].reshape(-1)
        perm = (pos2glob[:, None] * KB + np.arange(KB)[None, :]).reshape(-1)
        xtb = np.ascontiguousarray(x[b].T[:, perm])
        sub = np.arange(QG) // KB
        off = np.arange(QG) % KB
        glob_sub = sub if h == 0 else (sub ^ 1)
        qoff = glob_sub * KB + off
        kk = np.arange(KB)[:, None]
        m0 = (qoff[None, :] >= kk + h * KB).astype(np.float32)
        m1 = (qoff[None, :] >= kk + h * KB + 256).astype(np.float32)
        per_core.append(dict(perm=perm, in_map={
            "xt": xtb, "wq": np.asarray(Wq, np.float32) * SCALE,
            "wk": np.asarray(Wk, np.float32), "wv": np.asarray(Wv, np.float32),
            "mp": np.ascontiguousarray(np.concatenate([m0, m1], axis=1)),
        }))
    return per_core


def run(x, Wq, Wk, Wv, trace=False):
    from concourse.bass_utils import run_bass_kernel_spmd

    x = np.asarray(x, np.float32)
    nc = _get_program()
    per_core = _host_prepare(x, Wq, Wk, Wv)
    res = run_bass_kernel_spmd(
        nc, [pc["in_map"] for pc in per_core], core_ids=list(range(8)),
        trace=trace,
    )
    out = np.zeros((B, T, H), np.float32)
    for b in range(B):
        num = np.zeros((H, T), np.float64)
        den = np.zeros((1, T), np.float64)
        for c in (2 * b, 2 * b + 1):
            inv = np.argsort(per_core[c]["perm"])
            num += res.results[c]["ot"][:, inv]
            den += res.results[c]["dn"][:, inv]
        out[b] = (num / den).T
    return out, res


def kernel(x, Wq, Wk, Wv):
    out, _ = run(x, Wq, Wk, Wv, trace=False)
    return out



# revision 2
# speedup vs baseline: 1.4182x; 1.4182x over previous
"""Causal single-head attention (B=4, T=4096, C=1024, H=128) on 8 Trainium2
NeuronCores.

Sharding: 8 cores = 4 batches x 2 key-parity shards. Each core handles one
batch and the keys in every other 128-block (parity h = core % 2), computing
UN-normalized partial attention (numerator O^T and per-key exp sums) for ALL
4096 queries of its batch via unsafe softmax (scores are N(0,1)-bounded,
|s| < 10, so exp never overflows and max-subtraction is unnecessary; partial
results combine exactly by summation across the two cores of a batch).

SPMD uniformity trick: the program is identical on all cores; per-core
differences live entirely in the DATA. The host passes x[b].T with its
columns block-permuted so that this core's keys always sit at the EVEN
128-block positions, plus per-core causal mask tiles for the diagonal
blocks of each 512-query group. The host un-permutes the returned query
axis and combines: out = (O0 + O1) / (d0 + d1), transposed.

v2 (bf16): all SBUF operands are bf16 (PSUM accumulation stays f32):
  - halves x DMA traffic and SBUF footprint
  - DVE tensor ops run in 2x_1p mode (0.5 cyc/elem)
  - S psum tiles are [128, 1024] (2 banks, 2 key blocks): half the ACT
    instructions, one full-tile DVE accumulate per tile
  - the softmax denominator is NOT reduced on-chip: the per-group
    accumulated exp tile eacc [128, 1024] is DMA'd out raw and the host
    does the final 128x2-way sum (removes all ones-matmuls + [1,512]
    copies from the hot loop)
  - V^T->V transposes write 4 sub-blocks into ONE psum bank, one copy out

On-chip layout:
  S^T halves [128 keys, 512 queries] = matmul(lhsT=K^T block, rhs=Q^T group)
  E = exp(S^T) on ACT (PSUM->SBUF, bf16), diagonal tile masked on DVE
  O^T += matmul(lhsT=V block [k,H], rhs=E half)   accumulated in PSUM f32
  eacc += E on DVE (bf16), shipped to host per group
"""

import sys
import numpy as np

sys.path.insert(0, "/opt/trn_rl_repo")

B, T, C, H = 4, 4096, 1024, 128
KB = 128            # key block
QG = 512            # query group
NKB = T // KB       # 32 global key blocks
NQG = T // QG       # 8 query groups
NCH = C // 128      # 8 contraction chunks
NST = 4             # supertiles of 1024 positions
SCALE = float(H) ** -0.5

_prog_cache = {}


def _build_program():
    import concourse.mybir as mybir
    import concourse.tile as tile
    from concourse import bacc
    from concourse.masks import make_identity

    F32 = mybir.dt.float32
    BF16 = mybir.dt.bfloat16
    AF = mybir.ActivationFunctionType

    nc = bacc.Bacc()
    xt = nc.dram_tensor("xt", [C, T], BF16, kind="ExternalInput")
    wq = nc.dram_tensor("wq", [C, H], BF16, kind="ExternalInput")
    wk = nc.dram_tensor("wk", [C, H], BF16, kind="ExternalInput")
    wv = nc.dram_tensor("wv", [C, H], BF16, kind="ExternalInput")
    mp = nc.dram_tensor("mp", [KB, 2 * QG], BF16, kind="ExternalInput")
    ot = nc.dram_tensor("ot", [H, T], F32, kind="ExternalOutput")
    dn = nc.dram_tensor("dn", [KB, NQG * 2 * QG], BF16, kind="ExternalOutput")

    with tile.TileContext(nc) as tc:
        with (
            tc.tile_pool(name="singles", bufs=1) as singles,
            tc.tile_pool(name="persist", bufs=1) as persist,
            tc.tile_pool(name="xs0", bufs=8) as xsp0,
            tc.tile_pool(name="xsb", bufs=6) as xspb,
            tc.tile_pool(name="vst", bufs=2) as vstp,
            tc.tile_pool(name="epool", bufs=4) as epool,
            tc.tile_pool(name="dnp", bufs=2) as dnp,
            tc.tile_pool(name="outs", bufs=2) as outsp,
            tc.tile_pool(name="pproj", bufs=2, space="PSUM") as pproj,
            tc.tile_pool(name="ps", bufs=2, space="PSUM") as psp,
            tc.tile_pool(name="po", bufs=2, space="PSUM") as pop,
        ):
            # ---- constants ----
            w_sb = {}
            for name, w in (("wq", wq), ("wk", wk), ("wv", wv)):
                t_ = singles.tile([128, NCH, 128], BF16, tag=f"w_{name}")
                nc.sync.dma_start(out=t_, in_=w.rearrange("(c p) h -> p c h", p=128))
                w_sb[name] = t_
            mp_sb = singles.tile([KB, 2 * QG], BF16, tag="mp")
            nc.sync.dma_start(out=mp_sb, in_=mp[:])
            ident = singles.tile([128, 128], BF16, tag="ident")
            make_identity(nc, ident)

            qT = persist.tile([128, T], BF16, tag="qT")
            kT = persist.tile([128, T // 2], BF16, tag="kT")
            v_sb = persist.tile([128, T // 2], BF16, tag="v")

            # ---- stream in all of x^T up front ----
            # supertile 0 as 8 single-chunk DMAs (first chunk lands fast, PE
            # starts early); supertiles 1-3 as 2 batched 4-chunk DMAs each.
            xs_all = []
            xr = xt.rearrange("(c p) t -> p c t", p=128)
            xs = []
            for c in range(NCH):
                t_ = xsp0.tile([128, 1024], BF16, tag="xs0")
                nc.sync.dma_start(out=t_, in_=xr[:, c, 0:1024])
                xs.append(t_)
            xs_all.append(xs)
            for s in range(1, NST):
                lo = xspb.tile([128, 4, 1024], BF16, tag="xsb")
                nc.sync.dma_start(out=lo, in_=xr[:, 0:4, s * 1024:(s + 1) * 1024])
                hi = xspb.tile([128, 4, 1024], BF16, tag="xsb")
                nc.sync.dma_start(out=hi, in_=xr[:, 4:8, s * 1024:(s + 1) * 1024])
                xs_all.append([lo[:, c] for c in range(4)]
                              + [hi[:, c] for c in range(4)])

            def attention_group(j):
                nt = j + 1          # tiles of 2 key blocks each
                opsum = pop.tile([128, QG], F32, tag="opsum")
                eacc = dnp.tile([KB, 2 * QG], BF16, tag="eacc")
                qrhs = qT[:, j * QG:(j + 1) * QG]
                es = []

                def emit_pv(t):
                    e = es[t]
                    nc.tensor.matmul(
                        opsum, lhsT=v_sb[:, (2 * t) * KB:(2 * t + 1) * KB],
                        rhs=e[:, :QG], start=(t == 0), stop=False)
                    nc.tensor.matmul(
                        opsum, lhsT=v_sb[:, (2 * t + 1) * KB:(2 * t + 2) * KB],
                        rhs=e[:, QG:], start=False, stop=(t == nt - 1))

                for t in range(nt):
                    spsum = psp.tile([KB, 2 * QG], F32, tag="spsum")
                    nc.tensor.matmul(
                        spsum[:, :QG], lhsT=kT[:, (2 * t) * KB:(2 * t + 1) * KB],
                        rhs=qrhs, start=True, stop=True)
                    nc.tensor.matmul(
                        spsum[:, QG:], lhsT=kT[:, (2 * t + 1) * KB:(2 * t + 2) * KB],
                        rhs=qrhs, start=True, stop=True)
                    e = epool.tile([KB, 2 * QG], BF16, tag="e")
                    nc.scalar.activation(e, spsum, AF.Exp)
                    if t == nt - 1:
                        nc.vector.tensor_mul(e, e, mp_sb)
                    es.append(e)
                    if t == 0:
                        nc.vector.tensor_copy(eacc, e)
                    else:
                        nc.vector.tensor_add(eacc, eacc, e)
                    if t >= 1:
                        emit_pv(t - 1)      # PV lags one tile behind S/exp
                emit_pv(nt - 1)
                osb = outsp.tile([128, QG], F32, tag="osb")
                nc.vector.tensor_copy(osb, opsum)
                nc.sync.dma_start(out=ot[:, j * QG:(j + 1) * QG], in_=osb)
                nc.sync.dma_start(
                    out=dn[:, j * 2 * QG:(j + 1) * 2 * QG], in_=eacc)

            for s in range(NST):
                xs = xs_all[s]

                def keys_rhs(c):
                    # even 128-blocks of the supertile: cols 0-127, 256-383, ...
                    return xs[c].rearrange(
                        "p (u two b) -> p two u b", two=2, b=128)[:, 0]

                # K^T tile s  [128, 512]
                kpsum = pproj.tile([128, QG], F32, tag="proj")
                for c in range(NCH):
                    nc.tensor.matmul(kpsum, lhsT=w_sb["wk"][:, c], rhs=keys_rhs(c),
                                     start=(c == 0), stop=(c == NCH - 1))
                nc.vector.tensor_copy(kT[:, s * QG:(s + 1) * QG], kpsum)

                # V^T tile s -> transpose into V rows
                vpsum = pproj.tile([128, QG], F32, tag="proj")
                for c in range(NCH):
                    nc.tensor.matmul(vpsum, lhsT=w_sb["wv"][:, c], rhs=keys_rhs(c),
                                     start=(c == 0), stop=(c == NCH - 1))
                vstage = vstp.tile([128, QG], BF16, tag="vstage")
                nc.vector.tensor_copy(vstage, vpsum)
                vtp = pproj.tile([128, QG], BF16, tag="proj")
                for u in range(4):
                    nc.tensor.transpose(vtp[:, u * 128:(u + 1) * 128],
                                        vstage[:, u * 128:(u + 1) * 128], ident)
                nc.vector.tensor_copy(
                    v_sb[:, 4 * s * KB:(4 * s + 4) * KB], vtp)

                # Q^T tiles 2s, 2s+1 (scale folded into wq on host)
                for half in range(2):
                    tq = 2 * s + half
                    qpsum = pproj.tile([128, QG], F32, tag="proj")
                    for c in range(NCH):
                        nc.tensor.matmul(
                            qpsum, lhsT=w_sb["wq"][:, c],
                            rhs=xs[c][:, half * QG:(half + 1) * QG],
                            start=(c == 0), stop=(c == NCH - 1))
                    nc.vector.tensor_copy(qT[:, tq * QG:(tq + 1) * QG], qpsum)

                # attention for the two groups unlocked by this supertile
                attention_group(2 * s)
                attention_group(2 * s + 1)

    nc.finalize()
    return nc


def _get_program():
    if "nc" not in _prog_cache:
        _prog_cache["nc"] = _build_program()
    return _prog_cache["nc"]


def _host_prepare(x, Wq, Wk, Wv):
    """Per-core inputs. Core c: batch b=c//2, parity h=c%2."""
    from ml_dtypes import bfloat16

    wq16 = (np.asarray(Wq, np.float32) * SCALE).astype(bfloat16)
    wk16 = np.asarray(Wk, np.float32).astype(bfloat16)
    wv16 = np.asarray(Wv, np.float32).astype(bfloat16)
    per_core = []
    for c in range(8):
        b, h = c // 2, c % 2
        pos2glob = np.arange(NKB)
        if h == 1:
            pos2glob = pos2glob.reshape(-1, 2)[:, ::-1].reshape(-1)
        perm = (pos2glob[:, None] * KB + np.arange(KB)[None, :]).reshape(-1)
        xtb = np.ascontiguousarray(x[b].T[:, perm]).astype(bfloat16)
        sub = np.arange(QG) // KB
        off = np.arange(QG) % KB
        glob_sub = sub if h == 0 else (sub ^ 1)
        qoff = glob_sub * KB + off
        kk = np.arange(KB)[:, None]
        m0 = (qoff[None, :] >= kk + h * KB).astype(np.float32)
        m1 = (qoff[None, :] >= kk + h * KB + 256).astype(np.float32)
        mp_ = np.ascontiguousarray(
            np.concatenate([m0, m1], axis=1)).astype(bfloat16)
        per_core.append(dict(perm=perm, in_map={
            "xt": xtb, "wq": wq16, "wk": wk16, "wv": wv16, "mp": mp_,
        }))
    return per_core


def run(x, Wq, Wk, Wv, trace=False):
    from concourse.bass_utils import run_bass_kernel_spmd

    x = np.asarray(x, np.float32)
    nc = _get_program()
    per_core = _host_prepare(x, Wq, Wk, Wv)
    res = run_bass_kernel_spmd(
        nc, [pc["in_map"] for pc in per_core], core_ids=list(range(8)),
        trace=trace,
    )
    out = np.zeros((B, T, H), np.float32)
    for b in range(B):
        num = np.zeros((H, T), np.float64)
        den = np.zeros(T, np.float64)
        for c in (2 * b, 2 * b + 1):
            inv = np.argsort(per_core[c]["perm"])
            num += res.results[c]["ot"][:, inv]
            dnc = np.asarray(res.results[c]["dn"], np.float64)
            # [128, 8 groups, 2 halves, 512 q] -> per-query partial denom
            den_perm = dnc.reshape(KB, NQG, 2, QG).sum(axis=(0, 2)).reshape(-1)
            den += den_perm[inv]
        out[b] = (num / den[None, :]).T
    return out, res


def kernel(x, Wq, Wk, Wv):
    out, _ = run(x, Wq, Wk, Wv, trace=False)
    return out
